# revision 27
# baseline (speedup 1.0000x reference)
"""BoundaryLoss Trainium2 kernel — 8 NeuronCores, SPMD.

Sharding: core k -> batch b = k//2, channel half = k%2 (32 of 64 channels).
Each core:
  - receives a 5-plane zero-padded target halo stack for its batch
    (planes b-2..b+2; out-of-range planes are zeros, which are identity
    for dilation and absorbing for erosion -- exactly scipy border_value=0)
  - computes the radius-2 diamond dilation/erosion (== 2 iterations of the
    3-D cross) on device: bf16 max/min ops on DVE in flat-hw (128, 2048)
    tiles; w-shifts via strided APs; h-shifts split into the in-partition
    3/4 (direct AP offsets, no DMA) plus a small DMA'd partition-crossing
    sliver; neighbor batch planes pre-merged (max/min commute with shifts)
  - masked feature sums for 32 channels x {boundary, background}: features
    stream in as bf16 via casting SWDGE DMA (2 channels per 2 MB transfer),
    DVE does one broadcast-mask multiply per pair (2x perf mode), then the
    boundary products partition-reduce on TensorE (accumulating ones-matmul
    into a (1,512) PSUM tile, ScalarE finishes) while background products
    reduce on ScalarE directly (activation Copy + accum_out)
  - writes a (128, 68) f32 partial-sum tile to DRAM
Host: partition-reduces the partials, gathers across cores, and runs the
tiny (B,B) cosine/log epilogue in f32 numpy (bitwise-faithful NaN
semantics of the reference).

Measured on 8 axon-tunneled trn2 NeuronCores: ~162 us NEFF exec time
(per-core memory roofline: 33.5 MB feature read ~ 94 us @ 358 GB/s).
"""

import numpy as np

B, C, H, W = 4, 64, 512, 512
P = 128
FREE = (H * W) // P          # 2048
ROWS = FREE // W             # 4 h-rows per partition row
CPC = C // 2                 # channels per core
TEMPERATURE = 0.05
EPS = 1e-8

LAST_EXEC_NS = None
LAST_RESULTS = None

_NC_CACHE = {}


_MAX_WAITS = 1


def _patch_tile_drain():
    """This walrus build rejects instructions carrying more than ~1 sync
    wait ("Too many sync wait commands" in CoreV3 codegen). Hoist excess
    on_wait conditions onto same-engine NoOps emitted just before the
    instruction — both for Tile-scheduled instructions (_add_instruction)
    and for the kernel-tail drain (_drain_and_barrier)."""
    import concourse.mybir as mybir
    from concourse.tile import TileContext
    from concourse.vector_clock import ScopedClock

    if getattr(TileContext, "_drain_patched", False):
        return

    orig_add = TileContext._add_instruction

    def _add_instruction(self, inst):
        si = getattr(inst, "sync_info", None)
        maxw = 1 if isinstance(inst, mybir.InstDMACopy) else _MAX_WAITS
        if si is not None and si.on_wait and len(si.on_wait) > maxw:
            waits = list(si.on_wait)
            engine = inst.engine
            for i in range(0, len(waits) - maxw, maxw):
                chunk = waits[i : i + maxw]
                nop = mybir.InstNoOp(
                    name=self.nc.get_next_instruction_name(),
                    sync_info=mybir.SyncInfo(on_wait=chunk, on_update=[]),
                    bass_nofuse=True,
                    engine=engine,
                )
                orig_add(self, nop)
            si.on_wait = waits[len(waits) - maxw :]
        orig_add(self, inst)

    def _drain_and_barrier(self, tick_clock, wait_clock):
        nc = self.nc
        drain_inst = nc.sync.drain()
        wait_clock.add_sem_waits(
            drain_inst.ins, ScopedClock({None: tick_clock.global_clock})
        )
        si = drain_inst.ins.sync_info
        waits = list(si.on_wait) if si is not None and si.on_wait else []
        if len(waits) > _MAX_WAITS:
            si.on_wait = waits[:_MAX_WAITS]
            for w in waits[_MAX_WAITS:]:
                nop = nc.sync.nop()
                nop.ins.sync_info = mybir.SyncInfo(on_wait=[w], on_update=[])
        nc.all_engine_barrier()
        popped = nc._tile_sem_poison_stack.pop()
        assert popped is self._sem_poison
        nc.clear_and_free_semaphores(list(self.sems.allocated().values()))
        nc.all_engine_barrier()

    TileContext._add_instruction = _add_instruction
    TileContext._drain_and_barrier = _drain_and_barrier
    TileContext._drain_patched = True


def _build_nc():
    import concourse.bass as bass
    import concourse.mybir as mybir
    from concourse.tile import TileContext

    _patch_tile_drain()

    f32 = mybir.dt.float32
    bf16 = mybir.dt.bfloat16
    MAX = mybir.AluOpType.max
    MIN = mybir.AluOpType.min
    MULT = mybir.AluOpType.mult
    COPY = mybir.ActivationFunctionType.Copy

    nc = bass.Bass()
    feat = nc.declare_dram_parameter("feat", [CPC, P, FREE], f32, isOutput=False)
    tgt = nc.declare_dram_parameter("tgt", [5, P, FREE], bf16, isOutput=False)
    outp = nc.declare_dram_parameter("out", [P, 68], f32, isOutput=True)

    def v3(ap):
        # (P, FREE) -> (P, ROWS, W) view for w-edge-aware ops
        return ap.rearrange("p (r w) -> p r w", w=W)

    with (
        TileContext(nc) as tc,
        tc.tile_pool(name="masks", bufs=1) as pm,
        tc.tile_pool(name="feats", bufs=7) as pf,
        tc.tile_pool(name="scratch", bufs=3) as px,
        tc.tile_pool(name="psum", bufs=4, space="PSUM") as pp,
    ):
        tgt_ap = tgt[:]
        m = {}

        def load_tgt(j):
            t = pm.tile([P, FREE], bf16, tag=f"tgt{j}")
            nc.gpsimd.dma_start(out=t[:], in_=tgt_ap[j])
            m[j] = t

        def h_shifts(src, tagp, tagm, pool):
            """(hp, hm): src shifted by +1/-1 h-row, 0-padded borders.
            The two HWDGE rings (sync, scalar) each take one shift so the
            copies don't serialize on a single FIFO."""
            hp = pool.tile([P, FREE], bf16, tag=tagp)
            hm = pool.tile([P, FREE], bf16, tag=tagm)
            nc.vector.memset(hp[0:32, 0:W], 0.0)
            nc.gpsimd.dma_start(out=hp[:, W:FREE], in_=src[:, 0 : FREE - W])
            nc.gpsimd.dma_start(out=hp[1:P, 0:W], in_=src[0 : P - 1, FREE - W : FREE])
            nc.vector.memset(hm[96:P, FREE - W : FREE], 0.0)
            nc.gpsimd.dma_start(out=hm[:, 0 : FREE - W], in_=src[:, W:FREE])
            nc.gpsimd.dma_start(out=hm[0 : P - 1, FREE - W : FREE], in_=src[1:P, 0:W])
            return hp, hm

        def morph2d(dst, src, hp, hm, op):
            """dst = op over {src, w+-1 (0-padded), h+-1 (via hp/hm)}."""
            d3, s3 = v3(dst[:]), v3(src[:])
            nc.vector.tensor_tensor(
                out=d3[:, :, 0 : W - 1], in0=s3[:, :, 0 : W - 1],
                in1=s3[:, :, 1:W], op=op,
            )
            if op is MAX:
                nc.vector.tensor_copy(
                    out=d3[:, :, W - 1 : W], in_=s3[:, :, W - 1 : W]
                )
            else:
                nc.vector.memset(d3[:, :, W - 1 : W], 0.0)
            nc.vector.tensor_tensor(
                out=d3[:, :, 1:W], in0=d3[:, :, 1:W], in1=s3[:, :, 0 : W - 1], op=op
            )
            if op is MIN:
                nc.vector.memset(d3[:, :, 0:1], 0.0)
            nc.vector.tensor_tensor(out=dst[:], in0=dst[:], in1=hp[:], op=op)
            nc.vector.tensor_tensor(out=dst[:], in0=dst[:], in1=hm[:], op=op)

        def bmerge(dst, a, b_, op):
            nc.vector.tensor_tensor(out=dst[:], in0=dst[:], in1=a[:], op=op)
            nc.vector.tensor_tensor(out=dst[:], in0=dst[:], in1=b_[:], op=op)

        # ---- morphology, center-plane-first so iteration-2 shift DMAs
        # prefetch while DVE works on the neighbor planes ----
        # Radius-2 diamond decomposition: dil2 = max over |db|+|dh|+|dw|<=2.
        # W1 = w-radius-1 of m2, D = max(W1 ext to w-radius-2, h+-1 of W1,
        # h+-2 of m2, D1(m1), D1(m3), m0, m4); erosion mirrors with min.
        # SWDGE runs dma_starts in program order (head-of-line): emit every
        # shift DMA as soon as its source exists.

        def wmax1(dst, srct, op):
            """dst = op(src, w+-1 shifts of src), 0-padded."""
            d3, s3 = v3(dst[:]), v3(srct[:])
            nc.vector.tensor_tensor(
                out=d3[:, :, 0 : W - 1], in0=s3[:, :, 0 : W - 1],
                in1=s3[:, :, 1:W], op=op,
            )
            if op is MAX:
                nc.vector.tensor_copy(
                    out=d3[:, :, W - 1 : W], in_=s3[:, :, W - 1 : W]
                )
            else:
                nc.vector.memset(d3[:, :, W - 1 : W], 0.0)
            nc.vector.tensor_tensor(
                out=d3[:, :, 1:W], in0=d3[:, :, 1:W], in1=s3[:, :, 0 : W - 1], op=op
            )
            if op is MIN:
                nc.vector.memset(d3[:, :, 0:1], 0.0)

        def wext2(dst, w1, srct, op):
            """dst = op(w1, w+-2 shifts of src), 0-padded."""
            d3, w3, s3 = v3(dst[:]), v3(w1[:]), v3(srct[:])
            nc.vector.tensor_tensor(
                out=d3[:, :, 0 : W - 2], in0=w3[:, :, 0 : W - 2],
                in1=s3[:, :, 2:W], op=op,
            )
            nc.vector.tensor_copy(
                out=d3[:, :, W - 2 : W], in_=w3[:, :, W - 2 : W]
            )
            nc.vector.tensor_tensor(
                out=d3[:, :, 2:W], in0=d3[:, :, 2:W], in1=s3[:, :, 0 : W - 2], op=op
            )
            if op is MIN:
                nc.vector.memset(d3[:, :, 0:2], 0.0)
                nc.vector.memset(d3[:, :, W - 2 : W], 0.0)

        def cross_pieces(srct, tag, rows, engine=None):
            """The partition-crossing slivers of src shifted by +-rows:
            hb (for +shift: rows from the previous partition) and hb2
            (for -shift: rows from the next partition), 0-padded."""
            eng = engine if engine is not None else nc.gpsimd
            S = rows * W
            hb = pm.tile([P, S], bf16, tag=f"{tag}p")
            hb2 = pm.tile([P, S], bf16, tag=f"{tag}m")
            nc.vector.memset(hb[0:32, :], 0.0)
            eng.dma_start(out=hb[1:P, :], in_=srct[0 : P - 1, FREE - S : FREE])
            nc.vector.memset(hb2[96:P, :], 0.0)
            eng.dma_start(out=hb2[0 : P - 1, :], in_=srct[1:P, 0:S])
            return hb, hb2

        def hmerge_shift(dst, srct, pieces, op, rows=1):
            """dst = op(dst, h+-rows shifts of src). The in-partition 3/4
            comes straight from src via AP offsets; the crossing sliver
            from the small DMA'd piece tiles."""
            S = rows * W
            hb, hb2 = pieces
            nc.vector.tensor_tensor(
                out=dst[:, S:FREE], in0=dst[:, S:FREE], in1=srct[:, 0 : FREE - S],
                op=op,
            )
            nc.vector.tensor_tensor(out=dst[:, 0:S], in0=dst[:, 0:S], in1=hb[:], op=op)
            nc.vector.tensor_tensor(
                out=dst[:, 0 : FREE - S], in0=dst[:, 0 : FREE - S],
                in1=srct[:, S:FREE], op=op,
            )
            nc.vector.tensor_tensor(
                out=dst[:, FREE - S : FREE], in0=dst[:, FREE - S : FREE],
                in1=hb2[:], op=op,
            )

        load_tgt(2)
        W1d = pm.tile([P, FREE], bf16, tag="W1d")
        W1e = pm.tile([P, FREE], bf16, tag="W1e")
        wmax1(W1d, m[2], MAX)
        wmax1(W1e, m[2], MIN)
        pc2m = cross_pieces(m[2][:], "c2m", 2)
        load_tgt(1)
        load_tgt(3)
        # Neighbor planes merge BEFORE shifting (max/min commute with shifts)
        qd = pm.tile([P, FREE], bf16, tag="qd")
        nc.vector.tensor_tensor(out=qd[:], in0=m[1][:], in1=m[3][:], op=MAX)
        qe = pm.tile([P, FREE], bf16, tag="qe")
        nc.vector.tensor_tensor(out=qe[:], in0=m[1][:], in1=m[3][:], op=MIN)
        pcqd = cross_pieces(qd[:], "cqd", 1, engine=nc.sync)
        pcqe = cross_pieces(qe[:], "cqe", 1, engine=nc.sync)
        pcWd = cross_pieces(W1d[:], "cWd", 1, engine=nc.sync)
        pcWe = cross_pieces(W1e[:], "cWe", 1, engine=nc.sync)
        load_tgt(0)
        load_tgt(4)

        dil2 = pm.tile([P, FREE], bf16, tag="dil2")
        wext2(dil2, W1d, m[2], MAX)
        ero2 = pm.tile([P, FREE], bf16, tag="ero2")
        wext2(ero2, W1e, m[2], MIN)
        nc.vector.tensor_tensor(out=dil2[:], in0=dil2[:], in1=m[0][:], op=MAX)
        nc.vector.tensor_tensor(out=dil2[:], in0=dil2[:], in1=m[4][:], op=MAX)
        nc.vector.tensor_tensor(out=ero2[:], in0=ero2[:], in1=m[0][:], op=MIN)
        nc.vector.tensor_tensor(out=ero2[:], in0=ero2[:], in1=m[4][:], op=MIN)
        nd = pm.tile([P, FREE], bf16, tag="nd")
        wmax1(nd, qd, MAX)
        ne = pm.tile([P, FREE], bf16, tag="ne")
        wmax1(ne, qe, MIN)
        nt = pm.tile([P, FREE], bf16, tag="nt")
        nc.vector.tensor_scalar(
            out=nt[:], in0=m[2][:], scalar1=-1.0, scalar2=1.0,
            op0=MULT, op1=mybir.AluOpType.add,
        )

        hmerge_shift(dil2, W1d[:], pcWd, MAX, 1)
        hmerge_shift(dil2, m[2][:], pc2m, MAX, 2)
        hmerge_shift(ero2, W1e[:], pcWe, MIN, 1)
        hmerge_shift(ero2, m[2][:], pc2m, MIN, 2)
        hmerge_shift(nd, qd[:], pcqd, MAX, 1)
        nc.vector.tensor_tensor(out=dil2[:], in0=dil2[:], in1=nd[:], op=MAX)
        hmerge_shift(ne, qe[:], pcqe, MIN, 1)
        nc.vector.tensor_tensor(out=ero2[:], in0=ero2[:], in1=ne[:], op=MIN)

        # ---- boundary / background masks (0/1 exact in bf16) ----
        bnd = pm.tile([P, FREE], bf16, tag="bnd")
        nc.vector.tensor_tensor(
            out=bnd[:], in0=dil2[:], in1=ero2[:], op=mybir.AluOpType.subtract
        )
        bg = pm.tile([P, FREE], bf16, tag="bg")
        nc.vector.tensor_tensor(out=bg[:], in0=bnd[:], in1=nt[:], op=MIN)

        # ---- mask sums (ScalarE free-axis reduce) ----
        cols = pm.tile([P, 68], f32, tag="cols")
        nc.vector.memset(cols[:], 0.0)
        junk_bnd = px.tile([P, FREE], bf16, tag="junkw")
        nc.scalar.activation(
            out=junk_bnd[:], in_=bnd[:], func=COPY, accum_out=cols[:, 64:65]
        )
        junk_bg = px.tile([P, FREE], bf16, tag="junkw")
        nc.scalar.activation(
            out=junk_bg[:], in_=bg[:], func=COPY, accum_out=cols[:, 65:66]
        )

        # ---- masked feature sums: DVE bf16 multiply (2x mode), PE
        # ones-matmul partition-reduce into PSUM, ScalarE finishes the
        # (1, 512) free-axis reduce into cols ----
        ones = pm.tile([P, 1], bf16, tag="ones")
        nc.vector.memset(ones[:], 1.0)
        feat_ap = feat[:]

        def pair_product(ft2, mask):
            """(P, 2, FREE) product of two channels with one broadcast mask."""
            prod = px.tile([P, 2, FREE], bf16, tag="prod")
            mb = (
                mask[:]
                .rearrange("p (o f) -> p o f", o=1)
                .broadcast_to([P, 2, FREE])
            )
            nc.vector.tensor_tensor(out=prod[:], in0=ft2[:], in1=mb, op=MULT)
            return prod

        def reduce_pe(prodsrc, col):
            psr = pp.tile([1, 512], f32, tag="ps")
            for i in range(4):
                nc.tensor.matmul(
                    psr[:], ones[:], prodsrc[:, i * 512 : (i + 1) * 512],
                    start=(i == 0), stop=(i == 3),
                )
            junk = px.tile([1, 512], f32, tag="junk")
            nc.scalar.activation(
                out=junk[:], in_=psr[:], func=COPY, accum_out=cols[0:1, col : col + 1]
            )

        def reduce_act(prodsrc, col):
            junk = px.tile([P, FREE], bf16, tag="junkw")
            nc.scalar.activation(
                out=junk[:], in_=prodsrc, func=COPY, accum_out=cols[:, col : col + 1]
            )

        for c0 in range(0, CPC, 2):
            ft2 = pf.tile([P, 2, FREE], bf16, tag="feat")
            nc.gpsimd.dma_start(
                out=ft2[:], in_=feat_ap[c0 : c0 + 2].rearrange("c p f -> p c f")
            )
            prod_bnd = pair_product(ft2, bnd)
            prod_bg = pair_product(ft2, bg)
            for dc in range(2):
                c = c0 + dc
                reduce_pe(prod_bnd[:, dc], c)
                if c0 >= CPC - 4:
                    reduce_pe(prod_bg[:, dc], CPC + c)
                else:
                    reduce_act(prod_bg[:, dc], CPC + c)

        nc.sync.dma_start(out=outp[:], in_=cols[:])

    return nc


def _cosine(x, y):
    dot = np.sum(x * y, axis=-1)
    nx = np.sqrt(np.sum(x * x, axis=-1))
    ny = np.sqrt(np.sum(y * y, axis=-1))
    return dot / np.maximum(nx * ny, np.float32(EPS))


def kernel(features, target):
    global LAST_EXEC_NS, LAST_RESULTS
    from concourse import bass_utils

    feats = np.ascontiguousarray(features, dtype=np.float32)
    t = target.reshape(B, H, W).astype(np.float32)

    # 5-plane halo stacks per batch (zeros outside [0, B)), shipped bf16
    import ml_dtypes

    z = np.zeros((H, W), np.float32)
    stacks = []
    for b in range(B):
        planes = [t[b + d] if 0 <= b + d < B else z for d in (-2, -1, 0, 1, 2)]
        stacks.append(
            np.stack(planes).reshape(5, P, FREE).astype(ml_dtypes.bfloat16)
        )

    in_maps = []
    for k in range(8):
        b, half = k // 2, k % 2
        in_maps.append(
            {
                "feat": np.ascontiguousarray(
                    feats[b, half * CPC : (half + 1) * CPC].reshape(CPC, P, FREE)
                ),
                "tgt": stacks[b],
            }
        )

    if "nc" not in _NC_CACHE:
        _NC_CACHE["nc"] = _build_nc()
    res = bass_utils.run_bass_kernel_spmd(
        _NC_CACHE["nc"], in_maps, core_ids=list(range(8))
    )
    LAST_EXEC_NS = res.exec_time_ns
    LAST_RESULTS = res.results

    fs_bnd = np.zeros((B, C), np.float64)
    fs_bg = np.zeros((B, C), np.float64)
    ms_bnd = np.zeros((B,), np.float64)
    ms_bg = np.zeros((B,), np.float64)
    for k in range(8):
        b, half = k // 2, k % 2
        s = res.results[k]["out"].astype(np.float64).sum(axis=0)  # (68,)
        fs_bnd[b, half * CPC : (half + 1) * CPC] = s[:CPC]
        fs_bg[b, half * CPC : (half + 1) * CPC] = s[CPC : 2 * CPC]
        if half == 0:
            ms_bnd[b] = s[64]
            ms_bg[b] = s[65]

    # ---- epilogue in f32, mirroring the reference op-for-op ----
    fs_bnd = fs_bnd.astype(np.float32)
    fs_bg = fs_bg.astype(np.float32)
    ms_bnd_c = np.maximum(ms_bnd.astype(np.float32), np.float32(1.0))
    ms_bg_c = np.maximum(ms_bg.astype(np.float32), np.float32(1.0))

    bf = fs_bnd[None, :, :] / ms_bnd_c[:, None, None]  # (B,B,C)
    gf = fs_bg[None, :, :] / ms_bg_c[:, None, None]

    pos = _cosine(bf, bf)
    neg = _cosine(bf, gf)
    with np.errstate(all="ignore"):
        pos_loss = -np.log(np.exp(pos / np.float32(TEMPERATURE)))
        neg_loss = -np.log(np.float32(1.0) - np.exp(neg / np.float32(TEMPERATURE)))
        out = np.mean(pos_loss + neg_loss)
    return np.asarray(out, dtype=np.float32)


# revision 28
# speedup vs baseline: 1.3505x; 1.3505x over previous
"""BoundaryLoss Trainium2 kernel — 8 NeuronCores, SPMD.

Sharding: core k -> batch b = k//2, channel half = k%2 (32 of 64 channels).
Each core:
  - receives a 5-plane zero-padded target halo stack for its batch
    (planes b-2..b+2; out-of-range planes are zeros, which are identity
    for dilation and absorbing for erosion -- exactly scipy border_value=0)
  - computes the radius-2 diamond dilation/erosion (== 2 iterations of the
    3-D cross) on device: bf16 max/min ops on DVE in flat-hw (128, 2048)
    tiles; w-shifts via strided APs; h-shifts split into the in-partition
    3/4 (direct AP offsets, no DMA) plus a small DMA'd partition-crossing
    sliver; neighbor batch planes pre-merged (max/min commute with shifts)
  - masked feature sums for 32 channels x {boundary, background}: features
    stream in as bf16 via casting SWDGE DMA (2 channels per 2 MB transfer),
    DVE does one broadcast-mask multiply per pair (2x perf mode), then the
    boundary products partition-reduce on TensorE (accumulating ones-matmul
    into a (1,512) PSUM tile, ScalarE finishes) while background products
    reduce on ScalarE directly (activation Copy + accum_out)
  - writes a (128, 68) f32 partial-sum tile to DRAM
Host: partition-reduces the partials, gathers across cores, and runs the
tiny (B,B) cosine/log epilogue in f32 numpy (bitwise-faithful NaN
semantics of the reference).

Measured on 8 axon-tunneled trn2 NeuronCores: ~162 us NEFF exec time
(per-core memory roofline: 33.5 MB feature read ~ 94 us @ 358 GB/s).
"""

import numpy as np

B, C, H, W = 4, 64, 512, 512
P = 128
FREE = (H * W) // P          # 2048
ROWS = FREE // W             # 4 h-rows per partition row
CPC = C // 2                 # channels per core
TEMPERATURE = 0.05
EPS = 1e-8

LAST_EXEC_NS = None
LAST_RESULTS = None

_NC_CACHE = {}


_MAX_WAITS = 1


def _patch_tile_drain():
    """This walrus build rejects instructions carrying more than ~1 sync
    wait ("Too many sync wait commands" in CoreV3 codegen). Hoist excess
    on_wait conditions onto same-engine NoOps emitted just before the
    instruction — both for Tile-scheduled instructions (_add_instruction)
    and for the kernel-tail drain (_drain_and_barrier)."""
    import concourse.mybir as mybir
    from concourse.tile import TileContext
    from concourse.vector_clock import ScopedClock

    if getattr(TileContext, "_drain_patched", False):
        return

    orig_add = TileContext._add_instruction

    def _add_instruction(self, inst):
        si = getattr(inst, "sync_info", None)
        maxw = 1 if isinstance(inst, mybir.InstDMACopy) else _MAX_WAITS
        if si is not None and si.on_wait and len(si.on_wait) > maxw:
            waits = list(si.on_wait)
            engine = inst.engine
            for i in range(0, len(waits) - maxw, maxw):
                chunk = waits[i : i + maxw]
                nop = mybir.InstNoOp(
                    name=self.nc.get_next_instruction_name(),
                    sync_info=mybir.SyncInfo(on_wait=chunk, on_update=[]),
                    bass_nofuse=True,
                    engine=engine,
                )
                orig_add(self, nop)
            si.on_wait = waits[len(waits) - maxw :]
        orig_add(self, inst)

    def _drain_and_barrier(self, tick_clock, wait_clock):
        nc = self.nc
        drain_inst = nc.sync.drain()
        wait_clock.add_sem_waits(
            drain_inst.ins, ScopedClock({None: tick_clock.global_clock})
        )
        si = drain_inst.ins.sync_info
        waits = list(si.on_wait) if si is not None and si.on_wait else []
        if len(waits) > _MAX_WAITS:
            si.on_wait = waits[:_MAX_WAITS]
            for w in waits[_MAX_WAITS:]:
                nop = nc.sync.nop()
                nop.ins.sync_info = mybir.SyncInfo(on_wait=[w], on_update=[])
        nc.all_engine_barrier()
        popped = nc._tile_sem_poison_stack.pop()
        assert popped is self._sem_poison
        nc.clear_and_free_semaphores(list(self.sems.allocated().values()))
        nc.all_engine_barrier()

    TileContext._add_instruction = _add_instruction
    TileContext._drain_and_barrier = _drain_and_barrier
    TileContext._drain_patched = True


def _build_nc():
    import concourse.bass as bass
    import concourse.mybir as mybir
    from concourse.tile import TileContext

    _patch_tile_drain()

    f32 = mybir.dt.float32
    bf16 = mybir.dt.bfloat16
    MAX = mybir.AluOpType.max
    MIN = mybir.AluOpType.min
    MULT = mybir.AluOpType.mult
    COPY = mybir.ActivationFunctionType.Copy

    nc = bass.Bass()
    feat = nc.declare_dram_parameter("feat", [CPC, P, FREE], f32, isOutput=False)
    tgt = nc.declare_dram_parameter("tgt", [5, P, FREE], bf16, isOutput=False)
    outp = nc.declare_dram_parameter("out", [P, 68], f32, isOutput=True)

    def v3(ap):
        # (P, FREE) -> (P, ROWS, W) view for w-edge-aware ops
        return ap.rearrange("p (r w) -> p r w", w=W)

    with (
        TileContext(nc) as tc,
        tc.tile_pool(name="masks", bufs=1) as pm,
        tc.tile_pool(name="feats", bufs=7) as pf,
        tc.tile_pool(name="scratch", bufs=3) as px,
        tc.tile_pool(name="psum", bufs=4, space="PSUM") as pp,
    ):
        tgt_ap = tgt[:]
        m = {}

        def load_tgt(j):
            t = pm.tile([P, FREE], bf16, tag=f"tgt{j}")
            nc.gpsimd.dma_start(out=t[:], in_=tgt_ap[j])
            m[j] = t

        def h_shifts(src, tagp, tagm, pool):
            """(hp, hm): src shifted by +1/-1 h-row, 0-padded borders.
            The two HWDGE rings (sync, scalar) each take one shift so the
            copies don't serialize on a single FIFO."""
            hp = pool.tile([P, FREE], bf16, tag=tagp)
            hm = pool.tile([P, FREE], bf16, tag=tagm)
            nc.vector.memset(hp[0:32, 0:W], 0.0)
            nc.gpsimd.dma_start(out=hp[:, W:FREE], in_=src[:, 0 : FREE - W])
            nc.gpsimd.dma_start(out=hp[1:P, 0:W], in_=src[0 : P - 1, FREE - W : FREE])
            nc.vector.memset(hm[96:P, FREE - W : FREE], 0.0)
            nc.gpsimd.dma_start(out=hm[:, 0 : FREE - W], in_=src[:, W:FREE])
            nc.gpsimd.dma_start(out=hm[0 : P - 1, FREE - W : FREE], in_=src[1:P, 0:W])
            return hp, hm

        def morph2d(dst, src, hp, hm, op):
            """dst = op over {src, w+-1 (0-padded), h+-1 (via hp/hm)}."""
            d3, s3 = v3(dst[:]), v3(src[:])
            nc.vector.tensor_tensor(
                out=d3[:, :, 0 : W - 1], in0=s3[:, :, 0 : W - 1],
                in1=s3[:, :, 1:W], op=op,
            )
            if op is MAX:
                nc.vector.tensor_copy(
                    out=d3[:, :, W - 1 : W], in_=s3[:, :, W - 1 : W]
                )
            else:
                nc.vector.memset(d3[:, :, W - 1 : W], 0.0)
            nc.vector.tensor_tensor(
                out=d3[:, :, 1:W], in0=d3[:, :, 1:W], in1=s3[:, :, 0 : W - 1], op=op
            )
            if op is MIN:
                nc.vector.memset(d3[:, :, 0:1], 0.0)
            nc.vector.tensor_tensor(out=dst[:], in0=dst[:], in1=hp[:], op=op)
            nc.vector.tensor_tensor(out=dst[:], in0=dst[:], in1=hm[:], op=op)

        def bmerge(dst, a, b_, op):
            nc.vector.tensor_tensor(out=dst[:], in0=dst[:], in1=a[:], op=op)
            nc.vector.tensor_tensor(out=dst[:], in0=dst[:], in1=b_[:], op=op)

        # ---- morphology, center-plane-first so iteration-2 shift DMAs
        # prefetch while DVE works on the neighbor planes ----
        # Radius-2 diamond decomposition: dil2 = max over |db|+|dh|+|dw|<=2.
        # W1 = w-radius-1 of m2, D = max(W1 ext to w-radius-2, h+-1 of W1,
        # h+-2 of m2, D1(m1), D1(m3), m0, m4); erosion mirrors with min.
        # SWDGE runs dma_starts in program order (head-of-line): emit every
        # shift DMA as soon as its source exists.

        def wmax1(dst, srct, op):
            """dst = op(src, w+-1 shifts of src), 0-padded."""
            d3, s3 = v3(dst[:]), v3(srct[:])
            nc.vector.tensor_tensor(
                out=d3[:, :, 0 : W - 1], in0=s3[:, :, 0 : W - 1],
                in1=s3[:, :, 1:W], op=op,
            )
            if op is MAX:
                nc.vector.tensor_copy(
                    out=d3[:, :, W - 1 : W], in_=s3[:, :, W - 1 : W]
                )
            else:
                nc.vector.memset(d3[:, :, W - 1 : W], 0.0)
            nc.vector.tensor_tensor(
                out=d3[:, :, 1:W], in0=d3[:, :, 1:W], in1=s3[:, :, 0 : W - 1], op=op
            )
            if op is MIN:
                nc.vector.memset(d3[:, :, 0:1], 0.0)

        def wext2(dst, w1, srct, op):
            """dst = op(w1, w+-2 shifts of src), 0-padded."""
            d3, w3, s3 = v3(dst[:]), v3(w1[:]), v3(srct[:])
            nc.vector.tensor_tensor(
                out=d3[:, :, 0 : W - 2], in0=w3[:, :, 0 : W - 2],
                in1=s3[:, :, 2:W], op=op,
            )
            nc.vector.tensor_copy(
                out=d3[:, :, W - 2 : W], in_=w3[:, :, W - 2 : W]
            )
            nc.vector.tensor_tensor(
                out=d3[:, :, 2:W], in0=d3[:, :, 2:W], in1=s3[:, :, 0 : W - 2], op=op
            )
            if op is MIN:
                nc.vector.memset(d3[:, :, 0:2], 0.0)
                nc.vector.memset(d3[:, :, W - 2 : W], 0.0)

        def cross_pieces(srct, tag, rows):
            """The partition-crossing slivers of src shifted by +-rows:
            hb (for +shift: rows from the previous partition) and hb2
            (for -shift: rows from the next partition), 0-padded."""
            S = rows * W
            hb = pm.tile([P, S], bf16, tag=f"{tag}p")
            hb2 = pm.tile([P, S], bf16, tag=f"{tag}m")
            nc.vector.memset(hb[0:32, :], 0.0)
            nc.gpsimd.dma_start(out=hb[1:P, :], in_=srct[0 : P - 1, FREE - S : FREE])
            nc.vector.memset(hb2[96:P, :], 0.0)
            nc.gpsimd.dma_start(out=hb2[0 : P - 1, :], in_=srct[1:P, 0:S])
            return hb, hb2

        def hmerge_shift(dst, srct, pieces, op, rows=1):
            """dst = op(dst, h+-rows shifts of src). The in-partition 3/4
            comes straight from src via AP offsets; the crossing sliver
            from the small DMA'd piece tiles."""
            S = rows * W
            hb, hb2 = pieces
            nc.vector.tensor_tensor(
                out=dst[:, S:FREE], in0=dst[:, S:FREE], in1=srct[:, 0 : FREE - S],
                op=op,
            )
            nc.vector.tensor_tensor(out=dst[:, 0:S], in0=dst[:, 0:S], in1=hb[:], op=op)
            nc.vector.tensor_tensor(
                out=dst[:, 0 : FREE - S], in0=dst[:, 0 : FREE - S],
                in1=srct[:, S:FREE], op=op,
            )
            nc.vector.tensor_tensor(
                out=dst[:, FREE - S : FREE], in0=dst[:, FREE - S : FREE],
                in1=hb2[:], op=op,
            )

        load_tgt(2)
        W1d = pm.tile([P, FREE], bf16, tag="W1d")
        W1e = pm.tile([P, FREE], bf16, tag="W1e")
        wmax1(W1d, m[2], MAX)
        wmax1(W1e, m[2], MIN)
        pc2m = cross_pieces(m[2][:], "c2m", 2)
        load_tgt(1)
        load_tgt(3)
        # Neighbor planes merge BEFORE shifting (max/min commute with shifts)
        qd = pm.tile([P, FREE], bf16, tag="qd")
        nc.vector.tensor_tensor(out=qd[:], in0=m[1][:], in1=m[3][:], op=MAX)
        qe = pm.tile([P, FREE], bf16, tag="qe")
        nc.vector.tensor_tensor(out=qe[:], in0=m[1][:], in1=m[3][:], op=MIN)
        pcqd = cross_pieces(qd[:], "cqd", 1)
        pcqe = cross_pieces(qe[:], "cqe", 1)
        pcWd = cross_pieces(W1d[:], "cWd", 1)
        pcWe = cross_pieces(W1e[:], "cWe", 1)
        load_tgt(0)
        load_tgt(4)

        dil2 = pm.tile([P, FREE], bf16, tag="dil2")
        wext2(dil2, W1d, m[2], MAX)
        ero2 = pm.tile([P, FREE], bf16, tag="ero2")
        wext2(ero2, W1e, m[2], MIN)
        nc.vector.tensor_tensor(out=dil2[:], in0=dil2[:], in1=m[0][:], op=MAX)
        nc.vector.tensor_tensor(out=dil2[:], in0=dil2[:], in1=m[4][:], op=MAX)
        nc.vector.tensor_tensor(out=ero2[:], in0=ero2[:], in1=m[0][:], op=MIN)
        nc.vector.tensor_tensor(out=ero2[:], in0=ero2[:], in1=m[4][:], op=MIN)
        nd = pm.tile([P, FREE], bf16, tag="nd")
        wmax1(nd, qd, MAX)
        ne = pm.tile([P, FREE], bf16, tag="ne")
        wmax1(ne, qe, MIN)

        hmerge_shift(dil2, W1d[:], pcWd, MAX, 1)
        hmerge_shift(dil2, m[2][:], pc2m, MAX, 2)
        hmerge_shift(ero2, W1e[:], pcWe, MIN, 1)
        hmerge_shift(ero2, m[2][:], pc2m, MIN, 2)
        hmerge_shift(nd, qd[:], pcqd, MAX, 1)
        nc.vector.tensor_tensor(out=dil2[:], in0=dil2[:], in1=nd[:], op=MAX)
        hmerge_shift(ne, qe[:], pcqe, MIN, 1)
        nc.vector.tensor_tensor(out=ero2[:], in0=ero2[:], in1=ne[:], op=MIN)

        # ---- boundary / background masks (0/1 exact in bf16) ----
        bnd = pm.tile([P, FREE], bf16, tag="bnd")
        nc.vector.tensor_tensor(
            out=bnd[:], in0=dil2[:], in1=ero2[:], op=mybir.AluOpType.subtract
        )
        nt = pm.tile([P, FREE], bf16, tag="nt")
        nc.vector.tensor_scalar(
            out=nt[:], in0=m[2][:], scalar1=-1.0, scalar2=1.0,
            op0=MULT, op1=mybir.AluOpType.add,
        )
        bg = pm.tile([P, FREE], bf16, tag="bg")
        nc.vector.tensor_tensor(out=bg[:], in0=bnd[:], in1=nt[:], op=MIN)

        # ---- mask sums (ScalarE free-axis reduce) ----
        cols = pm.tile([P, 68], f32, tag="cols")
        nc.vector.memset(cols[:], 0.0)
        junk_bnd = px.tile([P, FREE], bf16, tag="junkw")
        nc.scalar.activation(
            out=junk_bnd[:], in_=bnd[:], func=COPY, accum_out=cols[:, 64:65]
        )
        junk_bg = px.tile([P, FREE], bf16, tag="junkw")
        nc.scalar.activation(
            out=junk_bg[:], in_=bg[:], func=COPY, accum_out=cols[:, 65:66]
        )

        # ---- masked feature sums: DVE bf16 multiply (2x mode), PE
        # ones-matmul partition-reduce into PSUM, ScalarE finishes the
        # (1, 512) free-axis reduce into cols ----
        ones = pm.tile([P, 1], bf16, tag="ones")
        nc.vector.memset(ones[:], 1.0)
        feat_ap = feat[:]

        def pair_product(ft2, mask):
            """(P, 2, FREE) product of two channels with one broadcast mask."""
            prod = px.tile([P, 2, FREE], bf16, tag="prod")
            mb = (
                mask[:]
                .rearrange("p (o f) -> p o f", o=1)
                .broadcast_to([P, 2, FREE])
            )
            nc.vector.tensor_tensor(out=prod[:], in0=ft2[:], in1=mb, op=MULT)
            return prod

        def reduce_pe(prodsrc, col):
            psr = pp.tile([1, 512], f32, tag="ps")
            for i in range(4):
                nc.tensor.matmul(
                    psr[:], ones[:], prodsrc[:, i * 512 : (i + 1) * 512],
                    start=(i == 0), stop=(i == 3),
                )
            junk = px.tile([1, 512], f32, tag="junk")
            nc.scalar.activation(
                out=junk[:], in_=psr[:], func=COPY, accum_out=cols[0:1, col : col + 1]
            )

        def reduce_act(prodsrc, col):
            junk = px.tile([P, FREE], bf16, tag="junkw")
            nc.scalar.activation(
                out=junk[:], in_=prodsrc, func=COPY, accum_out=cols[:, col : col + 1]
            )

        for c0 in range(0, CPC, 2):
            ft2 = pf.tile([P, 2, FREE], bf16, tag="feat")
            nc.gpsimd.dma_start(
                out=ft2[:], in_=feat_ap[c0 : c0 + 2].rearrange("c p f -> p c f")
            )
            prod_bnd = pair_product(ft2, bnd)
            prod_bg = pair_product(ft2, bg)
            for dc in range(2):
                c = c0 + dc
                reduce_pe(prod_bnd[:, dc], c)
                if c0 >= CPC - 4:
                    reduce_pe(prod_bg[:, dc], CPC + c)
                else:
                    reduce_act(prod_bg[:, dc], CPC + c)

        nc.sync.dma_start(out=outp[:], in_=cols[:])

    return nc


def _cosine(x, y):
    dot = np.sum(x * y, axis=-1)
    nx = np.sqrt(np.sum(x * x, axis=-1))
    ny = np.sqrt(np.sum(y * y, axis=-1))
    return dot / np.maximum(nx * ny, np.float32(EPS))


def kernel(features, target):
    global LAST_EXEC_NS, LAST_RESULTS
    from concourse import bass_utils

    feats = np.ascontiguousarray(features, dtype=np.float32)
    t = target.reshape(B, H, W).astype(np.float32)

    # 5-plane halo stacks per batch (zeros outside [0, B)), shipped bf16
    import ml_dtypes

    z = np.zeros((H, W), np.float32)
    stacks = []
    for b in range(B):
        planes = [t[b + d] if 0 <= b + d < B else z for d in (-2, -1, 0, 1, 2)]
        stacks.append(
            np.stack(planes).reshape(5, P, FREE).astype(ml_dtypes.bfloat16)
        )

    in_maps = []
    for k in range(8):
        b, half = k // 2, k % 2
        in_maps.append(
            {
                "feat": np.ascontiguousarray(
                    feats[b, half * CPC : (half + 1) * CPC].reshape(CPC, P, FREE)
                ),
                "tgt": stacks[b],
            }
        )

    if "nc" not in _NC_CACHE:
        _NC_CACHE["nc"] = _build_nc()
    res = bass_utils.run_bass_kernel_spmd(
        _NC_CACHE["nc"], in_maps, core_ids=list(range(8))
    )
    LAST_EXEC_NS = res.exec_time_ns
    LAST_RESULTS = res.results

    fs_bnd = np.zeros((B, C), np.float64)
    fs_bg = np.zeros((B, C), np.float64)
    ms_bnd = np.zeros((B,), np.float64)
    ms_bg = np.zeros((B,), np.float64)
    for k in range(8):
        b, half = k // 2, k % 2
        s = res.results[k]["out"].astype(np.float64).sum(axis=0)  # (68,)
        fs_bnd[b, half * CPC : (half + 1) * CPC] = s[:CPC]
        fs_bg[b, half * CPC : (half + 1) * CPC] = s[CPC : 2 * CPC]
        if half == 0:
            ms_bnd[b] = s[64]
            ms_bg[b] = s[65]

    # ---- epilogue in f32, mirroring the reference op-for-op ----
    fs_bnd = fs_bnd.astype(np.float32)
    fs_bg = fs_bg.astype(np.float32)
    ms_bnd_c = np.maximum(ms_bnd.astype(np.float32), np.float32(1.0))
    ms_bg_c = np.maximum(ms_bg.astype(np.float32), np.float32(1.0))

    bf = fs_bnd[None, :, :] / ms_bnd_c[:, None, None]  # (B,B,C)
    gf = fs_bg[None, :, :] / ms_bg_c[:, None, None]

    pos = _cosine(bf, bf)
    neg = _cosine(bf, gf)
    with np.errstate(all="ignore"):
        pos_loss = -np.log(np.exp(pos / np.float32(TEMPERATURE)))
        neg_loss = -np.log(np.float32(1.0) - np.exp(neg / np.float32(TEMPERATURE)))
        out = np.mean(pos_loss + neg_loss)
    return np.asarray(out, dtype=np.float32)


# revision 29
# speedup vs baseline: 1.4088x; 1.0432x over previous
"""BoundaryLoss Trainium2 kernel — 8 NeuronCores, SPMD.

Sharding: core k -> batch b = k//2, channel half = k%2 (32 of 64 channels).
Each core:
  - receives a 5-plane zero-padded target halo stack for its batch
    (planes b-2..b+2; out-of-range planes are zeros, which are identity
    for dilation and absorbing for erosion -- exactly scipy border_value=0)
  - computes the radius-2 diamond dilation/erosion (== 2 iterations of the
    3-D cross) on device: bf16 max/min ops on DVE in flat-hw (128, 2048)
    tiles; w-shifts via strided APs; h-shifts split into the in-partition
    3/4 (direct AP offsets, no DMA) plus a small DMA'd partition-crossing
    sliver; neighbor batch planes pre-merged (max/min commute with shifts)
  - masked feature sums for 32 channels x {boundary, background}: features
    stream in as bf16 via casting SWDGE DMA (2 channels per 2 MB transfer),
    DVE does one broadcast-mask multiply per pair (2x perf mode), then the
    boundary products partition-reduce on TensorE (accumulating ones-matmul
    into a (1,512) PSUM tile, ScalarE finishes) while background products
    reduce on ScalarE directly (activation Copy + accum_out)
  - writes a (128, 68) f32 partial-sum tile to DRAM
Host: partition-reduces the partials, gathers across cores, and runs the
tiny (B,B) cosine/log epilogue in f32 numpy (bitwise-faithful NaN
semantics of the reference).

Measured on 8 axon-tunneled trn2 NeuronCores: ~162 us NEFF exec time
(per-core memory roofline: 33.5 MB feature read ~ 94 us @ 358 GB/s).
"""

import numpy as np

B, C, H, W = 4, 64, 512, 512
P = 128
FREE = (H * W) // P          # 2048
ROWS = FREE // W             # 4 h-rows per partition row
CPC = C // 2                 # channels per core
TEMPERATURE = 0.05
EPS = 1e-8

LAST_EXEC_NS = None
LAST_RESULTS = None

_NC_CACHE = {}


_MAX_WAITS = 1


def _patch_tile_drain():
    """This walrus build rejects instructions carrying more than ~1 sync
    wait ("Too many sync wait commands" in CoreV3 codegen). Hoist excess
    on_wait conditions onto same-engine NoOps emitted just before the
    instruction — both for Tile-scheduled instructions (_add_instruction)
    and for the kernel-tail drain (_drain_and_barrier)."""
    import concourse.mybir as mybir
    from concourse.tile import TileContext
    from concourse.vector_clock import ScopedClock

    if getattr(TileContext, "_drain_patched", False):
        return

    orig_add = TileContext._add_instruction

    def _add_instruction(self, inst):
        si = getattr(inst, "sync_info", None)
        maxw = 1 if isinstance(inst, mybir.InstDMACopy) else _MAX_WAITS
        if si is not None and si.on_wait and len(si.on_wait) > maxw:
            waits = list(si.on_wait)
            engine = inst.engine
            for i in range(0, len(waits) - maxw, maxw):
                chunk = waits[i : i + maxw]
                nop = mybir.InstNoOp(
                    name=self.nc.get_next_instruction_name(),
                    sync_info=mybir.SyncInfo(on_wait=chunk, on_update=[]),
                    bass_nofuse=True,
                    engine=engine,
                )
                orig_add(self, nop)
            si.on_wait = waits[len(waits) - maxw :]
        orig_add(self, inst)

    def _drain_and_barrier(self, tick_clock, wait_clock):
        nc = self.nc
        drain_inst = nc.sync.drain()
        wait_clock.add_sem_waits(
            drain_inst.ins, ScopedClock({None: tick_clock.global_clock})
        )
        si = drain_inst.ins.sync_info
        waits = list(si.on_wait) if si is not None and si.on_wait else []
        if len(waits) > _MAX_WAITS:
            si.on_wait = waits[:_MAX_WAITS]
            for w in waits[_MAX_WAITS:]:
                nop = nc.sync.nop()
                nop.ins.sync_info = mybir.SyncInfo(on_wait=[w], on_update=[])
        nc.all_engine_barrier()
        popped = nc._tile_sem_poison_stack.pop()
        assert popped is self._sem_poison
        nc.clear_and_free_semaphores(list(self.sems.allocated().values()))
        nc.all_engine_barrier()

    TileContext._add_instruction = _add_instruction
    TileContext._drain_and_barrier = _drain_and_barrier
    TileContext._drain_patched = True


def _build_nc():
    import concourse.bass as bass
    import concourse.mybir as mybir
    from concourse.tile import TileContext

    _patch_tile_drain()

    f32 = mybir.dt.float32
    bf16 = mybir.dt.bfloat16
    MAX = mybir.AluOpType.max
    MIN = mybir.AluOpType.min
    MULT = mybir.AluOpType.mult
    COPY = mybir.ActivationFunctionType.Copy

    nc = bass.Bass()
    feat = nc.declare_dram_parameter("feat", [CPC, P, FREE], f32, isOutput=False)
    tgt = nc.declare_dram_parameter("tgt", [5, P, FREE], bf16, isOutput=False)
    outp = nc.declare_dram_parameter("out", [P, 68], f32, isOutput=True)

    def v3(ap):
        # (P, FREE) -> (P, ROWS, W) view for w-edge-aware ops
        return ap.rearrange("p (r w) -> p r w", w=W)

    with (
        TileContext(nc) as tc,
        tc.tile_pool(name="masks", bufs=1) as pm,
        tc.tile_pool(name="feats", bufs=4) as pf,
        tc.tile_pool(name="scratch", bufs=3) as px,
        tc.tile_pool(name="psum", bufs=4, space="PSUM") as pp,
    ):
        tgt_ap = tgt[:]
        m = {}

        def load_tgt(j):
            t = pm.tile([P, FREE], bf16, tag=f"tgt{j}")
            nc.gpsimd.dma_start(out=t[:], in_=tgt_ap[j])
            m[j] = t

        def h_shifts(src, tagp, tagm, pool):
            """(hp, hm): src shifted by +1/-1 h-row, 0-padded borders.
            The two HWDGE rings (sync, scalar) each take one shift so the
            copies don't serialize on a single FIFO."""
            hp = pool.tile([P, FREE], bf16, tag=tagp)
            hm = pool.tile([P, FREE], bf16, tag=tagm)
            nc.vector.memset(hp[0:32, 0:W], 0.0)
            nc.gpsimd.dma_start(out=hp[:, W:FREE], in_=src[:, 0 : FREE - W])
            nc.gpsimd.dma_start(out=hp[1:P, 0:W], in_=src[0 : P - 1, FREE - W : FREE])
            nc.vector.memset(hm[96:P, FREE - W : FREE], 0.0)
            nc.gpsimd.dma_start(out=hm[:, 0 : FREE - W], in_=src[:, W:FREE])
            nc.gpsimd.dma_start(out=hm[0 : P - 1, FREE - W : FREE], in_=src[1:P, 0:W])
            return hp, hm

        def morph2d(dst, src, hp, hm, op):
            """dst = op over {src, w+-1 (0-padded), h+-1 (via hp/hm)}."""
            d3, s3 = v3(dst[:]), v3(src[:])
            nc.vector.tensor_tensor(
                out=d3[:, :, 0 : W - 1], in0=s3[:, :, 0 : W - 1],
                in1=s3[:, :, 1:W], op=op,
            )
            if op is MAX:
                nc.vector.tensor_copy(
                    out=d3[:, :, W - 1 : W], in_=s3[:, :, W - 1 : W]
                )
            else:
                nc.vector.memset(d3[:, :, W - 1 : W], 0.0)
            nc.vector.tensor_tensor(
                out=d3[:, :, 1:W], in0=d3[:, :, 1:W], in1=s3[:, :, 0 : W - 1], op=op
            )
            if op is MIN:
                nc.vector.memset(d3[:, :, 0:1], 0.0)
            nc.vector.tensor_tensor(out=dst[:], in0=dst[:], in1=hp[:], op=op)
            nc.vector.tensor_tensor(out=dst[:], in0=dst[:], in1=hm[:], op=op)

        def bmerge(dst, a, b_, op):
            nc.vector.tensor_tensor(out=dst[:], in0=dst[:], in1=a[:], op=op)
            nc.vector.tensor_tensor(out=dst[:], in0=dst[:], in1=b_[:], op=op)

        # ---- morphology, center-plane-first so iteration-2 shift DMAs
        # prefetch while DVE works on the neighbor planes ----
        # Radius-2 diamond decomposition: dil2 = max over |db|+|dh|+|dw|<=2.
        # W1 = w-radius-1 of m2, D = max(W1 ext to w-radius-2, h+-1 of W1,
        # h+-2 of m2, D1(m1), D1(m3), m0, m4); erosion mirrors with min.
        # SWDGE runs dma_starts in program order (head-of-line): emit every
        # shift DMA as soon as its source exists.

        def wmax1(dst, srct, op):
            """dst = op(src, w+-1 shifts of src), 0-padded."""
            d3, s3 = v3(dst[:]), v3(srct[:])
            nc.vector.tensor_tensor(
                out=d3[:, :, 0 : W - 1], in0=s3[:, :, 0 : W - 1],
                in1=s3[:, :, 1:W], op=op,
            )
            if op is MAX:
                nc.vector.tensor_copy(
                    out=d3[:, :, W - 1 : W], in_=s3[:, :, W - 1 : W]
                )
            else:
                nc.vector.memset(d3[:, :, W - 1 : W], 0.0)
            nc.vector.tensor_tensor(
                out=d3[:, :, 1:W], in0=d3[:, :, 1:W], in1=s3[:, :, 0 : W - 1], op=op
            )
            if op is MIN:
                nc.vector.memset(d3[:, :, 0:1], 0.0)

        def wext2(dst, w1, srct, op):
            """dst = op(w1, w+-2 shifts of src), 0-padded."""
            d3, w3, s3 = v3(dst[:]), v3(w1[:]), v3(srct[:])
            nc.vector.tensor_tensor(
                out=d3[:, :, 0 : W - 2], in0=w3[:, :, 0 : W - 2],
                in1=s3[:, :, 2:W], op=op,
            )
            nc.vector.tensor_copy(
                out=d3[:, :, W - 2 : W], in_=w3[:, :, W - 2 : W]
            )
            nc.vector.tensor_tensor(
                out=d3[:, :, 2:W], in0=d3[:, :, 2:W], in1=s3[:, :, 0 : W - 2], op=op
            )
            if op is MIN:
                nc.vector.memset(d3[:, :, 0:2], 0.0)
                nc.vector.memset(d3[:, :, W - 2 : W], 0.0)

        def cross_pieces(srct, tag, rows):
            """The partition-crossing slivers of src shifted by +-rows:
            hb (for +shift: rows from the previous partition) and hb2
            (for -shift: rows from the next partition), 0-padded."""
            S = rows * W
            hb = pm.tile([P, S], bf16, tag=f"{tag}p")
            hb2 = pm.tile([P, S], bf16, tag=f"{tag}m")
            nc.vector.memset(hb[0:32, :], 0.0)
            nc.gpsimd.dma_start(out=hb[1:P, :], in_=srct[0 : P - 1, FREE - S : FREE])
            nc.vector.memset(hb2[96:P, :], 0.0)
            nc.gpsimd.dma_start(out=hb2[0 : P - 1, :], in_=srct[1:P, 0:S])
            return hb, hb2

        def hmerge_shift(dst, srct, pieces, op, rows=1):
            """dst = op(dst, h+-rows shifts of src). The in-partition 3/4
            comes straight from src via AP offsets; the crossing sliver
            from the small DMA'd piece tiles."""
            S = rows * W
            hb, hb2 = pieces
            nc.vector.tensor_tensor(
                out=dst[:, S:FREE], in0=dst[:, S:FREE], in1=srct[:, 0 : FREE - S],
                op=op,
            )
            nc.vector.tensor_tensor(out=dst[:, 0:S], in0=dst[:, 0:S], in1=hb[:], op=op)
            nc.vector.tensor_tensor(
                out=dst[:, 0 : FREE - S], in0=dst[:, 0 : FREE - S],
                in1=srct[:, S:FREE], op=op,
            )
            nc.vector.tensor_tensor(
                out=dst[:, FREE - S : FREE], in0=dst[:, FREE - S : FREE],
                in1=hb2[:], op=op,
            )

        load_tgt(2)
        W1d = pm.tile([P, FREE], bf16, tag="W1d")
        W1e = pm.tile([P, FREE], bf16, tag="W1e")
        wmax1(W1d, m[2], MAX)
        wmax1(W1e, m[2], MIN)
        pc2m = cross_pieces(m[2][:], "c2m", 2)
        load_tgt(1)
        load_tgt(3)
        # Neighbor planes merge BEFORE shifting (max/min commute with shifts)
        qd = pm.tile([P, FREE], bf16, tag="qd")
        nc.vector.tensor_tensor(out=qd[:], in0=m[1][:], in1=m[3][:], op=MAX)
        qe = pm.tile([P, FREE], bf16, tag="qe")
        nc.vector.tensor_tensor(out=qe[:], in0=m[1][:], in1=m[3][:], op=MIN)
        pcqd = cross_pieces(qd[:], "cqd", 1)
        pcqe = cross_pieces(qe[:], "cqe", 1)
        pcWd = cross_pieces(W1d[:], "cWd", 1)
        pcWe = cross_pieces(W1e[:], "cWe", 1)
        load_tgt(0)
        load_tgt(4)

        dil2 = pm.tile([P, FREE], bf16, tag="dil2")
        wext2(dil2, W1d, m[2], MAX)
        ero2 = pm.tile([P, FREE], bf16, tag="ero2")
        wext2(ero2, W1e, m[2], MIN)
        nc.vector.tensor_tensor(out=dil2[:], in0=dil2[:], in1=m[0][:], op=MAX)
        nc.vector.tensor_tensor(out=dil2[:], in0=dil2[:], in1=m[4][:], op=MAX)
        nc.vector.tensor_tensor(out=ero2[:], in0=ero2[:], in1=m[0][:], op=MIN)
        nc.vector.tensor_tensor(out=ero2[:], in0=ero2[:], in1=m[4][:], op=MIN)
        nd = pm.tile([P, FREE], bf16, tag="nd")
        wmax1(nd, qd, MAX)
        ne = pm.tile([P, FREE], bf16, tag="ne")
        wmax1(ne, qe, MIN)

        hmerge_shift(dil2, W1d[:], pcWd, MAX, 1)
        hmerge_shift(dil2, m[2][:], pc2m, MAX, 2)
        hmerge_shift(ero2, W1e[:], pcWe, MIN, 1)
        hmerge_shift(ero2, m[2][:], pc2m, MIN, 2)
        hmerge_shift(nd, qd[:], pcqd, MAX, 1)
        nc.vector.tensor_tensor(out=dil2[:], in0=dil2[:], in1=nd[:], op=MAX)
        hmerge_shift(ne, qe[:], pcqe, MIN, 1)
        nc.vector.tensor_tensor(out=ero2[:], in0=ero2[:], in1=ne[:], op=MIN)

        # ---- boundary / background masks (0/1 exact in bf16) ----
        bnd = pm.tile([P, FREE], bf16, tag="bnd")
        nc.vector.tensor_tensor(
            out=bnd[:], in0=dil2[:], in1=ero2[:], op=mybir.AluOpType.subtract
        )
        nt = pm.tile([P, FREE], bf16, tag="nt")
        nc.vector.tensor_scalar(
            out=nt[:], in0=m[2][:], scalar1=-1.0, scalar2=1.0,
            op0=MULT, op1=mybir.AluOpType.add,
        )
        bg = pm.tile([P, FREE], bf16, tag="bg")
        nc.vector.tensor_tensor(out=bg[:], in0=bnd[:], in1=nt[:], op=MIN)

        # ---- mask sums (ScalarE free-axis reduce) ----
        cols = pm.tile([P, 68], f32, tag="cols")
        nc.vector.memset(cols[:], 0.0)

        # ---- masked feature sums: DVE bf16 multiply (2x mode), PE
        # ones-matmul partition-reduce into PSUM, ScalarE finishes the
        # (1, 512) free-axis reduce into cols ----
        ones = pm.tile([P, 1], bf16, tag="ones")
        nc.vector.memset(ones[:], 1.0)
        feat_ap = feat[:]

        def mask_sum_pe(mask, col):
            psr = pp.tile([1, 512], f32, tag="ps")
            for i in range(4):
                nc.tensor.matmul(
                    psr[:], ones[:], mask[:, i * 512 : (i + 1) * 512],
                    start=(i == 0), stop=(i == 3),
                )
            junk = px.tile([1, 512], f32, tag="junk")
            nc.scalar.activation(
                out=junk[:], in_=psr[:], func=COPY, accum_out=cols[0:1, col : col + 1]
            )

        mask_sum_pe(bnd[:], 64)
        mask_sum_pe(bg[:], 65)

        def pair_product(ft2, mask):
            """(P, 2, FREE) product of two channels with one broadcast mask."""
            prod = px.tile([P, 2, FREE], bf16, tag="prod")
            mb = (
                mask[:]
                .rearrange("p (o f) -> p o f", o=1)
                .broadcast_to([P, 2, FREE])
            )
            nc.vector.tensor_tensor(out=prod[:], in0=ft2[:], in1=mb, op=MULT)
            return prod

        def reduce_pe(prodsrc, col):
            psr = pp.tile([1, 512], f32, tag="ps")
            for i in range(4):
                nc.tensor.matmul(
                    psr[:], ones[:], prodsrc[:, i * 512 : (i + 1) * 512],
                    start=(i == 0), stop=(i == 3),
                )
            junk = px.tile([1, 512], f32, tag="junk")
            nc.scalar.activation(
                out=junk[:], in_=psr[:], func=COPY, accum_out=cols[0:1, col : col + 1]
            )

        def reduce_act(prodsrc, col):
            junk = px.tile([P, FREE], bf16, tag="junkw")
            nc.scalar.activation(
                out=junk[:], in_=prodsrc, func=COPY, accum_out=cols[:, col : col + 1]
            )

        for c0 in range(0, CPC, 2):
            ft2 = pf.tile([P, 2, FREE], bf16, tag="feat")
            nc.gpsimd.dma_start(
                out=ft2[:], in_=feat_ap[c0 : c0 + 2].rearrange("c p f -> p c f")
            )
            prod_bnd = pair_product(ft2, bnd)
            prod_bg = pair_product(ft2, bg)
            for dc in range(2):
                c = c0 + dc
                reduce_pe(prod_bnd[:, dc], c)
                if c0 >= CPC - 12:
                    reduce_pe(prod_bg[:, dc], CPC + c)
                else:
                    reduce_act(prod_bg[:, dc], CPC + c)

        nc.sync.dma_start(out=outp[:], in_=cols[:])

    return nc


def _cosine(x, y):
    dot = np.sum(x * y, axis=-1)
    nx = np.sqrt(np.sum(x * x, axis=-1))
    ny = np.sqrt(np.sum(y * y, axis=-1))
    return dot / np.maximum(nx * ny, np.float32(EPS))


def kernel(features, target):
    global LAST_EXEC_NS, LAST_RESULTS
    from concourse import bass_utils

    feats = np.ascontiguousarray(features, dtype=np.float32)
    t = target.reshape(B, H, W).astype(np.float32)

    # 5-plane halo stacks per batch (zeros outside [0, B)), shipped bf16
    import ml_dtypes

    z = np.zeros((H, W), np.float32)
    stacks = []
    for b in range(B):
        planes = [t[b + d] if 0 <= b + d < B else z for d in (-2, -1, 0, 1, 2)]
        stacks.append(
            np.stack(planes).reshape(5, P, FREE).astype(ml_dtypes.bfloat16)
        )

    in_maps = []
    for k in range(8):
        b, half = k // 2, k % 2
        in_maps.append(
            {
                "feat": np.ascontiguousarray(
                    feats[b, half * CPC : (half + 1) * CPC].reshape(CPC, P, FREE)
                ),
                "tgt": stacks[b],
            }
        )

    if "nc" not in _NC_CACHE:
        _NC_CACHE["nc"] = _build_nc()
    res = bass_utils.run_bass_kernel_spmd(
        _NC_CACHE["nc"], in_maps, core_ids=list(range(8))
    )
    LAST_EXEC_NS = res.exec_time_ns
    LAST_RESULTS = res.results

    fs_bnd = np.zeros((B, C), np.float64)
    fs_bg = np.zeros((B, C), np.float64)
    ms_bnd = np.zeros((B,), np.float64)
    ms_bg = np.zeros((B,), np.float64)
    for k in range(8):
        b, half = k // 2, k % 2
        s = res.results[k]["out"].astype(np.float64).sum(axis=0)  # (68,)
        fs_bnd[b, half * CPC : (half + 1) * CPC] = s[:CPC]
        fs_bg[b, half * CPC : (half + 1) * CPC] = s[CPC : 2 * CPC]
        if half == 0:
            ms_bnd[b] = s[64]
            ms_bg[b] = s[65]

    # ---- epilogue in f32, mirroring the reference op-for-op ----
    fs_bnd = fs_bnd.astype(np.float32)
    fs_bg = fs_bg.astype(np.float32)
    ms_bnd_c = np.maximum(ms_bnd.astype(np.float32), np.float32(1.0))
    ms_bg_c = np.maximum(ms_bg.astype(np.float32), np.float32(1.0))

    bf = fs_bnd[None, :, :] / ms_bnd_c[:, None, None]  # (B,B,C)
    gf = fs_bg[None, :, :] / ms_bg_c[:, None, None]

    pos = _cosine(bf, bf)
    neg = _cosine(bf, gf)
    with np.errstate(all="ignore"):
        pos_loss = -np.log(np.exp(pos / np.float32(TEMPERATURE)))
        neg_loss = -np.log(np.float32(1.0) - np.exp(neg / np.float32(TEMPERATURE)))
        out = np.mean(pos_loss + neg_loss)
    return np.asarray(out, dtype=np.float32)
